# revision 33
# baseline (speedup 1.0000x reference)
"""AlphaPermutationLayer Trainium2 kernel.

out[i, j] = sum_k softmax(alpha/T)[k] * (perm[k, i] == j),  N=2048, K=64.

Strategy: shard OUTPUT ROWS across the 8 cores (each output row depends only
on perm[:, row] and alpha, so no collective is needed).  Per core (256 rows):
digit-split j = jq*64 + jf (jq in [0,32), jf in [0,64)); pair column i
couples two rows r0/r1 (one per k-half h); per row
    out_r[jq, jf] = sum_k e_k * (ph[k,r] == jq) * (pl[k,r] == jf)
with e_k = 2*exp(alpha_k/T).  The e-scaled high-digit one-hot A is the
stationary operand (bf16), the unscaled low-digit one-hot B the moving one;
two 64-contraction matmuls per pair (one per k-half, distinct PE tile
positions) accumulate into PSUM; softmax normalization 1/(2S) is applied
per-partition at the ACT PSUM->SBUF evacuation (tolerance is 2e-2, so a
single bf16 pass is plenty).

Row remap r = pi*128 + h*64 + b*8 + s makes each PSUM bank's 32 rows
DRAM-contiguous per (pi, h) quarter, so the output leaves in 16 DMAs of
128KB (3-dim APs) over the sync and gpsimd rings. Builds are chunked in 4
groups of 32 pair columns and pipelined against the plx input DMA, the
matmul stream, evacuation, and the output DMAs.
"""

import os
import sys

sys.path.insert(0, "/opt/trn_rl_repo")

import numpy as np

N = 2048
K = 64
NCORES = 8
ROWS = N // NCORES          # 256 rows per core
DP = 32                     # stationary digit width (jq), psum partitions per row
DF = 64                     # moving digit width (jf), psum free per row
CW = 32                     # i-chunk width (pair columns per build chunk)
NCHUNK = 128 // CW
PREWARM = int(os.environ.get("KERNEL_PREWARM", "80"))

LAST_EXEC_NS = None
LAST_RESULTS = None

_cached = {}


def _build_bass():
    import concourse.tile as tile
    from concourse import bacc, mybir
    from concourse.bass import _add_dep_helper

    fp32 = mybir.dt.float32
    bf16 = mybir.dt.bfloat16
    i16 = mybir.dt.int16

    nc = bacc.Bacc()

    ph_ext = nc.declare_dram_parameter("ph", [128, 128], i16, isOutput=False)
    pl_ext = nc.declare_dram_parameter("pl", [128, 128], i16, isOutput=False)
    at_ext = nc.declare_dram_parameter("at", [128, 2], fp32, isOutput=False)
    ifx_ext = nc.declare_dram_parameter("ifx", [128, DF * CW], i16, isOutput=False)
    out_ext = nc.declare_dram_parameter("out", [ROWS, N], fp32, isOutput=True)

    with tile.TileContext(nc) as tc:
        with (
            tc.tile_pool(name="sbuf", bufs=1) as sb,
            tc.tile_pool(name="stage", bufs=3) as stp,
            tc.tile_pool(name="smax_psum", bufs=1, space="PSUM") as psmax,
            tc.tile_pool(name="psum", bufs=7, space="PSUM") as pp,
        ):
            # ---- input DMAs -------------------------------------------------
            # ifx[p, c, i] = c is the shared compare table: B uses the full
            # c range [0, 64) as the jf iota, A reuses rows [0, 32) as jq.
            ph_t = sb.tile([128, 128], i16)
            pl_t = sb.tile([128, 128], i16)
            at_t = sb.tile([128, 2], fp32)
            ifx_t = sb.tile([128, DF, CW], i16)
            ifx_view = ifx_ext[:].rearrange("p (c i) -> p c i", i=CW)
            # sync ring leads with the A half of ifx (c < 32); the B half
            # follows on the other two rings
            nc.sync.dma_start(out=ifx_t[:, 0:32], in_=ifx_view[:, 0:32])
            nc.scalar.dma_start(out=at_t[:], in_=at_ext[:])
            nc.scalar.dma_start(out=ifx_t[:, 32:48], in_=ifx_view[:, 32:48])
            nc.gpsimd.dma_start(out=ph_t[:], in_=ph_ext[:])
            nc.gpsimd.dma_start(out=pl_t[:], in_=pl_ext[:])
            nc.gpsimd.dma_start(out=ifx_t[:, 48:64], in_=ifx_view[:, 48:64])

            # ---- softmax head (late normalization) --------------------------
            # e = 2*exp(a/T) (ln2 bias); partition sum of the k-duplicated e
            # gives 4S; evacuation scale rs = 2/(4S) = 1/(2S) turns the
            # accumulated 2S-weighted sum into the true softmax-weighted sum.
            e_t = sb.tile([128, 1], fp32)
            ln2_t = sb.tile([128, 1], fp32)
            prime_t = sb.tile([128, 1], fp32)
            ones_col = sb.tile([128, 1], fp32)
            ones_row = sb.tile([1, 128], fp32)
            s_sb = sb.tile([1, 1], fp32)
            rs_t = sb.tile([128, 1], fp32)
            nc.vector.memset(ln2_t[:], float(np.log(2.0)))
            nc.vector.memset(ones_col[:], 1.0)
            nc.vector.memset(ones_row[:], 2.0)
            # dep-free ACT op hoists the activation-table load off the path
            nc.scalar.activation(
                out=prime_t[:], in_=ln2_t[:],
                func=mybir.ActivationFunctionType.Exp,
            )
            nc.scalar.activation(
                out=e_t[:],
                in_=at_t[:, 0:1],
                func=mybir.ActivationFunctionType.Exp,
                bias=ln2_t[:],
            )

            # HAM pre-warm: dep-free PE work so the clock gate ramps before
            # the real matmul stream (overwritten by the real sum below).
            sum_ps = psmax.tile([1, 1], fp32, tag="smax")
            for _ in range(PREWARM):
                nc.tensor.matmul(
                    sum_ps[:], lhsT=ones_col[:], rhs=ones_col[:],
                    start=True, stop=True,
                )
            # 4S = sum_p e ; r = 1/(4S) ; rs = broadcast(2 * r) = 1/(2S)
            nc.tensor.matmul(
                sum_ps[:], lhsT=e_t[:], rhs=ones_col[:], start=True, stop=True
            )

            # ---- one-hot builds, chunked (all DVE, 2x/4x modes) -------------
            # A: a_t[p=(h,k), c, i] = (ph[p, i] == c), then as_t = a_t * e
            # B (f-major): b_t[p, f, i] = (pl[p, i] == f)  (unscaled, moving;
            # the matmul rhs reads it i-strided)
            a_t = sb.tile([128, DP, 128], bf16)
            as_t = sb.tile([128, DP, 128], bf16)
            b_t = sb.tile([128, DF, 128], bf16)
            as0_i = None
            for g in range(NCHUNK):
                ic = slice(g * CW, (g + 1) * CW)
                nc.vector.tensor_tensor(
                    out=a_t[:, :, ic],
                    in0=ph_t[:, ic].unsqueeze(1).to_broadcast([128, DP, CW]),
                    in1=ifx_t[:, 0:DP, :],
                    op=mybir.AluOpType.is_equal,
                )
                nc.vector.tensor_tensor(
                    out=b_t[:, :, ic],
                    in0=pl_t[:, ic].unsqueeze(1).to_broadcast([128, DF, CW]),
                    in1=ifx_t[:],
                    op=mybir.AluOpType.is_equal,
                )
                as_i = nc.vector.tensor_scalar(
                    out=as_t[:, :, ic],
                    in0=a_t[:, :, ic],
                    scalar1=e_t[:],
                    scalar2=None,
                    op0=mybir.AluOpType.mult,
                )
                if g == 0:
                    as0_i = as_i

            # the softmax reciprocal must not be hoisted ahead of the
            # chunk-0 builds (it blocks the DVE on the PE partition-sum,
            # which sits behind the prewarm stream)
            rec_i = nc.vector.reciprocal(out=s_sb[:], in_=sum_ps[:])
            _add_dep_helper(rec_i.ins, as0_i.ins, sync=False,
                            reason="recip after chunk0 builds")
            rb_ps = psmax.tile([128, 1], fp32, tag="smax")
            nc.tensor.matmul(
                rb_ps[:], lhsT=ones_row[:], rhs=s_sb[:], start=True, stop=True
            )
            nc.scalar.activation(
                out=rs_t[:], in_=rb_ps[:],
                func=mybir.ActivationFunctionType.Copy,
            )

            # ---- per-pair matmuls + evacuation + store ----------------------
            # psum partition q' = 64*pi + 32*h + jq; slab row r = 128*pi +
            # 64*h + 8*b + s (host remap) so each (pi, h) quarter of two
            # consecutive banks is one DRAM-contiguous [32, 16, 64] region.
            oview = out_ext[:].rearrange(
                "(pp hh bs) (q f) -> pp hh q bs f", pp=2, hh=2, bs=64, q=DP, f=DF
            )
            Copy = mybir.ActivationFunctionType.Copy
            stage2 = None
            for b in range(8):
                bank = pp.tile([128, 8, DF], mybir.dt.float32, tag="bank")
                # s outer / pi / h inner: consecutive matmuls rotate over the
                # four PE tile positions (64h, 64pi+32h) so LDWEIGHTS of the
                # next pair overlaps the in-flight matmul.
                for s in range(8):
                    for pi in range(2):
                        i = b * 16 + pi * 8 + s
                        for h in range(2):
                            kp = slice(64 * h, 64 * h + 64)
                            q0 = 64 * pi + 32 * h
                            nc.tensor.matmul(
                                bank[q0 : q0 + 32, s],
                                lhsT=as_t[kp, :, i],
                                rhs=b_t[kp, :, i],
                                start=True,
                                stop=True,
                                tile_position=(64 * h, q0),
                            )
                if b % 2 == 0:
                    stage2 = stp.tile([128, 16, DF], mybir.dt.float32, tag="stage")
                if b in (5, 7):
                    # tail evacuations ride the DVE (free after the builds)
                    # so they don't serialize behind ACT's evac chain
                    nc.vector.tensor_scalar(
                        out=stage2[:, 8 * (b % 2) : 8 * (b % 2) + 8, :],
                        in0=bank[:],
                        scalar1=rs_t[:],
                        scalar2=None,
                        op0=mybir.AluOpType.mult,
                    )
                else:
                    nc.scalar.activation(
                        out=stage2[:, 8 * (b % 2) : 8 * (b % 2) + 8, :],
                        in_=bank[:],
                        func=Copy,
                        scale=rs_t[:],
                    )
                if b % 2 == 1:
                    g = b // 2
                    bs = slice(16 * g, 16 * g + 16)
                    for pi in range(2):
                        for h in range(2):
                            # rotate the three rings across the 16 quarters,
                            # giving the (slower) SWDGE ring the lightest share
                            ring = [nc.sync, nc.scalar, nc.gpsimd, nc.sync,
                                    nc.scalar, nc.sync, nc.gpsimd, nc.scalar,
                                    nc.sync, nc.scalar, nc.gpsimd, nc.sync,
                                    nc.scalar, nc.sync, nc.gpsimd, nc.scalar,
                                    ][4 * g + 2 * pi + h]
                            q0 = 64 * pi + 32 * h
                            ring.dma_start(
                                out=oview[pi, h, :, bs, :],
                                in_=stage2[q0 : q0 + 32, :, :],
                            )
    if not nc.is_finalized():
        nc.finalize()
    return nc


def _prep_inputs(alpha_weights, perm_vectors, temperature):
    a = np.asarray(alpha_weights, dtype=np.float32).reshape(K)
    T = np.asarray(temperature, dtype=np.float32).reshape(())
    perm = np.asarray(perm_vectors).astype(np.int64).reshape(K, N)
    ph = (perm >> 6).astype(np.int16)
    pl = (perm & 63).astype(np.int16)
    at = np.empty((128, 2), dtype=np.float32)
    at[:, 0] = np.concatenate([a, a]) / T
    at[:, 1] = T
    ifx = np.broadcast_to(
        np.arange(DF, dtype=np.int16)[None, :, None], (128, DF, CW)
    ).reshape(128, -1).copy()
    # pair column i = b*16 + pi*8 + s holds slab rows r(h) = pi*128 + h*64 +
    # b*8 + s (the remap that makes bank quarters DRAM-contiguous)
    i_idx = np.arange(128)
    b_i, pi_i, s_i = i_idx // 16, (i_idx % 16) // 8, i_idx % 8
    cols = pi_i * 128 + b_i * 8 + s_i              # h=0 rows; h=1 adds 64
    in_maps = []
    for cid in range(NCORES):
        base = cid * ROWS
        ph_c = np.empty((128, 128), dtype=np.int16)
        pl_c = np.empty((128, 128), dtype=np.int16)
        for h in range(2):
            ph_c[64 * h : 64 * h + 64, :] = ph[:, base + cols + 64 * h]
            pl_c[64 * h : 64 * h + 64, :] = pl[:, base + cols + 64 * h]
        in_maps.append(
            {
                "ph": ph_c,
                "pl": pl_c,
                "at": at,
                "ifx": ifx,
            }
        )
    return in_maps


def _install_ntff_hook():
    """Provide antenv.axon_hooks (missing in this image) so that
    run_bass_kernel_spmd(trace=True) can capture NTFF profiles via the
    axon PJRT .so (same mechanism as trn_agent_boot.trn_boot)."""
    import contextlib
    import ctypes
    import types

    try:
        from antenv.axon_hooks import get_axon_ntff_profile_hook  # noqa: F401

        return True
    except ImportError:
        pass
    so_path = "/opt/axon/libaxon_pjrt.so"
    if not os.path.exists(so_path):
        return False
    lib = ctypes.CDLL(so_path)
    if not hasattr(lib, "axon_start_nrt_profile"):
        return False
    lib.axon_start_nrt_profile.argtypes = [
        ctypes.POINTER(ctypes.c_int64),
        ctypes.c_size_t,
    ]
    lib.axon_start_nrt_profile.restype = ctypes.c_int64
    lib.axon_stop_nrt_profile.argtypes = [ctypes.c_char_p]
    lib.axon_stop_nrt_profile.restype = ctypes.c_int64

    @contextlib.contextmanager
    def _hook(output_dir, device_ids):
        import jax

        jax.devices()
        if device_ids:
            ids = (ctypes.c_int64 * len(device_ids))(*device_ids)
            rc = lib.axon_start_nrt_profile(ids, len(device_ids))
        else:
            rc = lib.axon_start_nrt_profile(None, 0)
        if rc != 0:
            raise RuntimeError(f"axon_start_nrt_profile rc={rc}")
        try:
            yield
        finally:
            n = lib.axon_stop_nrt_profile(str(output_dir).encode())
            print(f"ntff profile: {n} file(s) written to {output_dir}")

    import antenv

    mod = types.ModuleType("antenv.axon_hooks")
    mod.get_axon_ntff_profile_hook = lambda: _hook
    mod.set_axon_ntff_profile_hook = lambda h: None
    sys.modules["antenv.axon_hooks"] = mod
    antenv.axon_hooks = mod
    return True


def kernel(alpha_weights, perm_vectors, temperature):
    global LAST_EXEC_NS, LAST_RESULTS
    from concourse.bass_utils import run_bass_kernel_spmd

    if "nc" not in _cached:
        _cached["nc"] = _build_bass()
    nc = _cached["nc"]
    in_maps = _prep_inputs(alpha_weights, perm_vectors, temperature)
    core_ids = list(range(NCORES))
    trace = os.environ.get("KERNEL_TRACE", "0") == "1"
    if trace:
        trace = _install_ntff_hook()
    try:
        res = run_bass_kernel_spmd(nc, in_maps, core_ids, trace=trace)
    except Exception:
        if not trace:
            raise
        res = run_bass_kernel_spmd(nc, in_maps, core_ids, trace=False)
    LAST_EXEC_NS = res.exec_time_ns
    LAST_RESULTS = res
    out = np.concatenate([res.results[c]["out"] for c in range(NCORES)], axis=0)
    return out.astype(np.float32)


if __name__ == "__main__":
    rng = np.random.default_rng(0)
    a = rng.standard_normal(K).astype(np.float32)
    perm = np.stack([rng.permutation(N) for _ in range(K)]).astype(np.int64)
    T = np.ones((), np.float32)
    out = kernel(a, perm, T)
    # numpy reference
    al = np.exp(a / T - (a / T).max())
    al /= al.sum()
    exp = np.zeros((N, N), np.float32)
    np.add.at(exp, (np.broadcast_to(np.arange(N), (K, N)), perm), al[:, None])
    print("max abs err:", np.abs(out - exp).max(), "max ref:", np.abs(exp).max())
    print("exec ns:", LAST_EXEC_NS)


# revision 35
# speedup vs baseline: 1.1706x; 1.1706x over previous
"""AlphaPermutationLayer Trainium2 kernel.

out[i, j] = sum_k softmax(alpha/T)[k] * (perm[k, i] == j),  N=2048, K=64.

Strategy: shard OUTPUT ROWS across the 8 cores (each output row depends only
on perm[:, row] and alpha, so no collective is needed).  Per core (256 rows):
digit-split j = jq*64 + jf (jq in [0,32), jf in [0,64)); pair column i
couples two rows r0/r1 (one per k-half h); per row
    out_r[jq, jf] = sum_k e_k * (ph[k,r] == jq) * (pl[k,r] == jf)
with e_k = 2*exp(alpha_k/T).  The e-scaled high-digit one-hot A is the
stationary operand (bf16), the unscaled low-digit one-hot B the moving one;
two 64-contraction matmuls per pair (one per k-half, distinct PE tile
positions) accumulate into PSUM; softmax normalization 1/(2S) is applied
per-partition at the ACT PSUM->SBUF evacuation (tolerance is 2e-2, so a
single bf16 pass is plenty).

Row remap r = pi*128 + h*64 + b*8 + s makes each PSUM bank's 32 rows
DRAM-contiguous per (pi, h) quarter, so the output leaves in 16 DMAs of
128KB (3-dim APs) over the sync and gpsimd rings. Builds are chunked in 4
groups of 32 pair columns and pipelined against the plx input DMA, the
matmul stream, evacuation, and the output DMAs.
"""

import os
import sys

sys.path.insert(0, "/opt/trn_rl_repo")

import numpy as np

N = 2048
K = 64
NCORES = 8
ROWS = N // NCORES          # 256 rows per core
DP = 32                     # stationary digit width (jq), psum partitions per row
DF = 64                     # moving digit width (jf), psum free per row
CW = 32                     # i-chunk width (pair columns per build chunk)
NCHUNK = 128 // CW
PREWARM = int(os.environ.get("KERNEL_PREWARM", "80"))

LAST_EXEC_NS = None
LAST_RESULTS = None

_cached = {}


def _build_bass():
    import concourse.tile as tile
    from concourse import bacc, mybir
    from concourse.bass import _add_dep_helper

    fp32 = mybir.dt.float32
    bf16 = mybir.dt.bfloat16
    i16 = mybir.dt.int16

    nc = bacc.Bacc()

    ph_ext = nc.declare_dram_parameter("ph", [128, 128], i16, isOutput=False)
    pl_ext = nc.declare_dram_parameter("pl", [128, 128], i16, isOutput=False)
    at_ext = nc.declare_dram_parameter("at", [128, 2], fp32, isOutput=False)
    ifx_ext = nc.declare_dram_parameter("ifx", [128, DF * CW], i16, isOutput=False)
    out_ext = nc.declare_dram_parameter("out", [ROWS, N], fp32, isOutput=True)

    with tile.TileContext(nc) as tc:
        with (
            tc.tile_pool(name="sbuf", bufs=1) as sb,
            tc.tile_pool(name="stage", bufs=3) as stp,
            tc.tile_pool(name="smax_psum", bufs=1, space="PSUM") as psmax,
            tc.tile_pool(name="psum", bufs=7, space="PSUM") as pp,
        ):
            # ---- input DMAs -------------------------------------------------
            # ifx[p, c, i] = c is the shared compare table: B uses the full
            # c range [0, 64) as the jf iota, A reuses rows [0, 32) as jq.
            ph_t = sb.tile([128, 128], i16)
            pl_t = sb.tile([128, 128], i16)
            at_t = sb.tile([128, 2], fp32)
            ifx_t = sb.tile([128, DF, CW], i16)
            ifx_view = ifx_ext[:].rearrange("p (c i) -> p c i", i=CW)
            # sync ring leads with the A half of ifx (c < 32); the B half
            # follows on the other two rings
            nc.sync.dma_start(out=ifx_t[:, 0:32], in_=ifx_view[:, 0:32])
            nc.scalar.dma_start(out=at_t[:], in_=at_ext[:])
            nc.scalar.dma_start(out=ifx_t[:, 32:48], in_=ifx_view[:, 32:48])
            nc.gpsimd.dma_start(out=ph_t[:], in_=ph_ext[:])
            nc.gpsimd.dma_start(out=pl_t[:], in_=pl_ext[:])
            nc.gpsimd.dma_start(out=ifx_t[:, 48:64], in_=ifx_view[:, 48:64])

            # ---- softmax head (late normalization) --------------------------
            # e = 2*exp(a/T) (ln2 bias); partition sum of the k-duplicated e
            # gives 4S; evacuation scale rs = 2/(4S) = 1/(2S) turns the
            # accumulated 2S-weighted sum into the true softmax-weighted sum.
            e_t = sb.tile([128, 1], fp32)
            ln2_t = sb.tile([128, 1], fp32)
            prime_t = sb.tile([128, 1], fp32)
            ones_col = sb.tile([128, 1], fp32)
            ones_row = sb.tile([1, 128], fp32)
            s_sb = sb.tile([1, 1], fp32)
            rs_t = sb.tile([128, 1], fp32)
            nc.vector.memset(ln2_t[:], float(np.log(2.0)))
            nc.vector.memset(ones_col[:], 1.0)
            nc.vector.memset(ones_row[:], 2.0)
            # dep-free ACT op hoists the activation-table load off the path
            nc.scalar.activation(
                out=prime_t[:], in_=ln2_t[:],
                func=mybir.ActivationFunctionType.Exp,
            )
            nc.scalar.activation(
                out=e_t[:],
                in_=at_t[:, 0:1],
                func=mybir.ActivationFunctionType.Exp,
                bias=ln2_t[:],
            )

            # HAM pre-warm: dep-free PE work so the clock gate ramps before
            # the real matmul stream (overwritten by the real sum below).
            sum_ps = psmax.tile([1, 1], fp32, tag="smax")
            for _ in range(PREWARM):
                nc.tensor.matmul(
                    sum_ps[:], lhsT=ones_col[:], rhs=ones_col[:],
                    start=True, stop=True,
                )
            # 4S = sum_p e ; r = 1/(4S) ; rs = broadcast(2 * r) = 1/(2S)
            nc.tensor.matmul(
                sum_ps[:], lhsT=e_t[:], rhs=ones_col[:], start=True, stop=True
            )

            # ---- one-hot builds, chunked (all DVE, 2x/4x modes) -------------
            # A: a_t[p=(h,k), c, i] = (ph[p, i] == c), then as_t = a_t * e
            # B (f-major): b_t[p, f, i] = (pl[p, i] == f)  (unscaled, moving;
            # the matmul rhs reads it i-strided)
            a_t = sb.tile([128, DP, 128], bf16)
            as_t = sb.tile([128, DP, 128], bf16)
            b_t = sb.tile([128, DF, 128], bf16)

            def build_a(g):
                ic = slice(g * CW, (g + 1) * CW)
                nc.vector.tensor_tensor(
                    out=a_t[:, :, ic],
                    in0=ph_t[:, ic].unsqueeze(1).to_broadcast([128, DP, CW]),
                    in1=ifx_t[:, 0:DP, :],
                    op=mybir.AluOpType.is_equal,
                )

            def build_b(g):
                ic = slice(g * CW, (g + 1) * CW)
                nc.vector.tensor_tensor(
                    out=b_t[:, :, ic],
                    in0=pl_t[:, ic].unsqueeze(1).to_broadcast([128, DF, CW]),
                    in1=ifx_t[:],
                    op=mybir.AluOpType.is_equal,
                )

            def scale_a(g):
                ic = slice(g * CW, (g + 1) * CW)
                nc.vector.tensor_scalar(
                    out=as_t[:, :, ic],
                    in0=a_t[:, :, ic],
                    scalar1=e_t[:],
                    scalar2=None,
                    op0=mybir.AluOpType.mult,
                )

            # hand-scheduled DVE order: A-side work (whose inputs land
            # first) fills the wait for the B half of ifx; the softmax
            # reciprocal rides after the first scale so it can't stall
            # the build stream on the PE partition-sum.
            build_a(0)
            build_a(1)
            scale_a(0)
            nc.vector.reciprocal(out=s_sb[:], in_=sum_ps[:])
            build_b(0)
            scale_a(1)
            build_b(1)
            build_a(2)
            build_b(2)
            scale_a(2)
            build_a(3)
            build_b(3)
            scale_a(3)

            rb_ps = psmax.tile([128, 1], fp32, tag="smax")
            nc.tensor.matmul(
                rb_ps[:], lhsT=ones_row[:], rhs=s_sb[:], start=True, stop=True
            )
            nc.scalar.activation(
                out=rs_t[:], in_=rb_ps[:],
                func=mybir.ActivationFunctionType.Copy,
            )

            # ---- per-pair matmuls + evacuation + store ----------------------
            # psum partition q' = 64*pi + 32*h + jq; slab row r = 128*pi +
            # 64*h + 8*b + s (host remap) so each (pi, h) quarter of two
            # consecutive banks is one DRAM-contiguous [32, 16, 64] region.
            oview = out_ext[:].rearrange(
                "(pp hh bs) (q f) -> pp hh q bs f", pp=2, hh=2, bs=64, q=DP, f=DF
            )
            Copy = mybir.ActivationFunctionType.Copy
            stage2 = None
            for b in range(8):
                bank = pp.tile([128, 8, DF], mybir.dt.float32, tag="bank")
                # s outer / pi / h inner: consecutive matmuls rotate over the
                # four PE tile positions (64h, 64pi+32h) so LDWEIGHTS of the
                # next pair overlaps the in-flight matmul.
                for s in range(8):
                    for pi in range(2):
                        i = b * 16 + pi * 8 + s
                        for h in range(2):
                            kp = slice(64 * h, 64 * h + 64)
                            q0 = 64 * pi + 32 * h
                            nc.tensor.matmul(
                                bank[q0 : q0 + 32, s],
                                lhsT=as_t[kp, :, i],
                                rhs=b_t[kp, :, i],
                                start=True,
                                stop=True,
                                tile_position=(64 * h, q0),
                            )
                if b % 2 == 0:
                    stage2 = stp.tile([128, 16, DF], mybir.dt.float32, tag="stage")
                nc.scalar.activation(
                    out=stage2[:, 8 * (b % 2) : 8 * (b % 2) + 8, :],
                    in_=bank[:],
                    func=Copy,
                    scale=rs_t[:],
                )
                if b % 2 == 1:
                    g = b // 2
                    bs = slice(16 * g, 16 * g + 16)
                    for pi in range(2):
                        for h in range(2):
                            # rotate the three rings across the 16 quarters,
                            # giving the (slower) SWDGE ring the lightest share
                            ring = [nc.sync, nc.scalar, nc.gpsimd, nc.sync,
                                    nc.scalar, nc.sync, nc.gpsimd, nc.scalar,
                                    nc.sync, nc.scalar, nc.gpsimd, nc.sync,
                                    nc.scalar, nc.sync, nc.gpsimd, nc.scalar,
                                    ][4 * g + 2 * pi + h]
                            q0 = 64 * pi + 32 * h
                            ring.dma_start(
                                out=oview[pi, h, :, bs, :],
                                in_=stage2[q0 : q0 + 32, :, :],
                            )
    if not nc.is_finalized():
        nc.finalize()
    return nc


def _prep_inputs(alpha_weights, perm_vectors, temperature):
    a = np.asarray(alpha_weights, dtype=np.float32).reshape(K)
    T = np.asarray(temperature, dtype=np.float32).reshape(())
    perm = np.asarray(perm_vectors).astype(np.int64).reshape(K, N)
    ph = (perm >> 6).astype(np.int16)
    pl = (perm & 63).astype(np.int16)
    at = np.empty((128, 2), dtype=np.float32)
    at[:, 0] = np.concatenate([a, a]) / T
    at[:, 1] = T
    ifx = np.broadcast_to(
        np.arange(DF, dtype=np.int16)[None, :, None], (128, DF, CW)
    ).reshape(128, -1).copy()
    # pair column i = b*16 + pi*8 + s holds slab rows r(h) = pi*128 + h*64 +
    # b*8 + s (the remap that makes bank quarters DRAM-contiguous)
    i_idx = np.arange(128)
    b_i, pi_i, s_i = i_idx // 16, (i_idx % 16) // 8, i_idx % 8
    cols = pi_i * 128 + b_i * 8 + s_i              # h=0 rows; h=1 adds 64
    in_maps = []
    for cid in range(NCORES):
        base = cid * ROWS
        ph_c = np.empty((128, 128), dtype=np.int16)
        pl_c = np.empty((128, 128), dtype=np.int16)
        for h in range(2):
            ph_c[64 * h : 64 * h + 64, :] = ph[:, base + cols + 64 * h]
            pl_c[64 * h : 64 * h + 64, :] = pl[:, base + cols + 64 * h]
        in_maps.append(
            {
                "ph": ph_c,
                "pl": pl_c,
                "at": at,
                "ifx": ifx,
            }
        )
    return in_maps


def _install_ntff_hook():
    """Provide antenv.axon_hooks (missing in this image) so that
    run_bass_kernel_spmd(trace=True) can capture NTFF profiles via the
    axon PJRT .so (same mechanism as trn_agent_boot.trn_boot)."""
    import contextlib
    import ctypes
    import types

    try:
        from antenv.axon_hooks import get_axon_ntff_profile_hook  # noqa: F401

        return True
    except ImportError:
        pass
    so_path = "/opt/axon/libaxon_pjrt.so"
    if not os.path.exists(so_path):
        return False
    lib = ctypes.CDLL(so_path)
    if not hasattr(lib, "axon_start_nrt_profile"):
        return False
    lib.axon_start_nrt_profile.argtypes = [
        ctypes.POINTER(ctypes.c_int64),
        ctypes.c_size_t,
    ]
    lib.axon_start_nrt_profile.restype = ctypes.c_int64
    lib.axon_stop_nrt_profile.argtypes = [ctypes.c_char_p]
    lib.axon_stop_nrt_profile.restype = ctypes.c_int64

    @contextlib.contextmanager
    def _hook(output_dir, device_ids):
        import jax

        jax.devices()
        if device_ids:
            ids = (ctypes.c_int64 * len(device_ids))(*device_ids)
            rc = lib.axon_start_nrt_profile(ids, len(device_ids))
        else:
            rc = lib.axon_start_nrt_profile(None, 0)
        if rc != 0:
            raise RuntimeError(f"axon_start_nrt_profile rc={rc}")
        try:
            yield
        finally:
            n = lib.axon_stop_nrt_profile(str(output_dir).encode())
            print(f"ntff profile: {n} file(s) written to {output_dir}")

    import antenv

    mod = types.ModuleType("antenv.axon_hooks")
    mod.get_axon_ntff_profile_hook = lambda: _hook
    mod.set_axon_ntff_profile_hook = lambda h: None
    sys.modules["antenv.axon_hooks"] = mod
    antenv.axon_hooks = mod
    return True


def kernel(alpha_weights, perm_vectors, temperature):
    global LAST_EXEC_NS, LAST_RESULTS
    from concourse.bass_utils import run_bass_kernel_spmd

    if "nc" not in _cached:
        _cached["nc"] = _build_bass()
    nc = _cached["nc"]
    in_maps = _prep_inputs(alpha_weights, perm_vectors, temperature)
    core_ids = list(range(NCORES))
    trace = os.environ.get("KERNEL_TRACE", "0") == "1"
    if trace:
        trace = _install_ntff_hook()
    try:
        res = run_bass_kernel_spmd(nc, in_maps, core_ids, trace=trace)
    except Exception:
        if not trace:
            raise
        res = run_bass_kernel_spmd(nc, in_maps, core_ids, trace=False)
    LAST_EXEC_NS = res.exec_time_ns
    LAST_RESULTS = res
    out = np.concatenate([res.results[c]["out"] for c in range(NCORES)], axis=0)
    return out.astype(np.float32)


if __name__ == "__main__":
    rng = np.random.default_rng(0)
    a = rng.standard_normal(K).astype(np.float32)
    perm = np.stack([rng.permutation(N) for _ in range(K)]).astype(np.int64)
    T = np.ones((), np.float32)
    out = kernel(a, perm, T)
    # numpy reference
    al = np.exp(a / T - (a / T).max())
    al /= al.sum()
    exp = np.zeros((N, N), np.float32)
    np.add.at(exp, (np.broadcast_to(np.arange(N), (K, N)), perm), al[:, None])
    print("max abs err:", np.abs(out - exp).max(), "max ref:", np.abs(exp).max())
    print("exec ns:", LAST_EXEC_NS)
